# revision 1
# baseline (speedup 1.0000x reference)
"""Trainium2 Bass kernel for nn_DevConv_52896817217994 (gnn_message_passing).

Math reduction:
  s = nodes @ W_theta                       (per-node scalar, [N])
  proj[e] = s[row[e]] - s[col[e]]           (linearity of the projection)
  d[e] = |proj[e]|
  segmax[n] = max_{e: row[e]=n} d[e]
            = max(s[n] - min_{e} s[col[e]], max_{e} s[col[e]] - s[n])  (clamped at 0)
  out = 0.5 * (prev + mean(W_phi) * max(segmax, 0))

Distribution strategy: edges are routed to the 8 cores by destination-node
(row) range -- each core owns a contiguous 12500-node slice and all edges
incident to it as a padded CSR.  Within a core, nodes are rank-sorted by
degree and packed into strips of (128*sp_g nodes) x (W_g slots) chosen by a
small DP so the padding inflation stays ~5%; strip shapes are shared by all
cores (global widths).  Pad slots duplicate a real neighbor, empty nodes
self-loop (which yields exactly 0).  The device computes the projection
(fp16), per-node segmented min/max reduces, and the final blend; outputs are
disjoint node slices (no collective); the host un-permutes the degree sort.

HW-measured design choices (in-NEFF loop benchmarks):
  - gpsimd elementwise is ~7 ns/elem on real HW (cost model says 0.9):
    ALL arithmetic goes to DVE (fp16 2x) which hides under the DMA bound.
  - many small DMAs run at ~110 GB/s: the whole rectangle is packed into ONE
    DRAM tensor, fetched by a few ~0.5 MB DMAs alternated over the two HWDGE
    rings (sync / scalar), and stays resident in SBUF (~20 KB/partition).

Device inputs per core (all produced by pure layout/permutation on host):
  rect : [128, 3*TOT] f16  neighbor planes, grouped [RX|RY|RZ] per DMA group
  wth  : [128, 4]  f32     W_theta (replicated)
  cst  : [128, 523] f32    W_phi(128)|W_theta(3)|nx|ny|nz|prev (rank order)
Output:
  y    : [128, 98] f32     per-node scores in flat-slot order
"""

import sys

if "/opt/trn_rl_repo" not in sys.path:
    sys.path.insert(0, "/opt/trn_rl_repo")

import numpy as np

N_NODES = 100000
N_EDGES = 3200000
N_CORES = 8
NPC = N_NODES // N_CORES  # 12500 nodes per core
P = 128                   # partitions
SP = 98                   # node slots per partition (98*128 = 12544 >= 12500)
NPAD = P * SP             # padded nodes per core
CST_W = 128 + 3 + 4 * SP  # packed const width
MAX_STRIPS = 8
GROUP_ELEMS = 900         # target per-plane elems per DMA group

_prog_cache = {}
LAST_RESULTS = None


def _legalize_waits(nc):
    """The walrus codegen path used under axon embeds at most ONE sync wait
    per instruction (setupSyncWait asserts otherwise).  Tile can emit several
    (e.g. a DMA wait plus a same-engine RAW wait).  Split the extras onto
    same-engine NoOp carriers placed immediately before the instruction."""
    import concourse.mybir as mybir

    for f in nc.m.functions:
        for bb in f.blocks:
            out = []
            changed = False
            for ins in bb.instructions:
                si = ins.sync_info
                if si is not None and si.on_wait and len(si.on_wait) > 1:
                    changed = True
                    for w in si.on_wait[:-1]:
                        nop = mybir.InstNoOp(
                            name=f"WS-{nc.next_id()}",
                            engine=ins.engine,
                            bass_nofuse=True,
                            text_hint="wait_split",
                            sync_info=mybir.SyncInfo(on_wait=[w], on_update=[]),
                        )
                        nc.inst_map[nop.name] = nop
                        out.append(nop)
                    ins.sync_info = mybir.SyncInfo(
                        on_wait=[si.on_wait[-1]], on_update=si.on_update)
                out.append(ins)
            if changed:
                bb.instructions = out
    return nc


def _choose_strips(degmax_sp):
    """Partition the SP degree-rank blocks into <=MAX_STRIPS contiguous strips
    minimizing total padded slots.  degmax_sp[b] = width needed by block b
    (non-increasing).  Returns [(sp_g, W_g), ...]."""
    n = len(degmax_sp)
    W = [max(int(w), 2) for w in degmax_sp]

    INF = float("inf")
    dp = [[INF] * (n + 1) for _ in range(MAX_STRIPS + 1)]
    nxt = [[None] * (n + 1) for _ in range(MAX_STRIPS + 1)]
    for g in range(MAX_STRIPS + 1):
        dp[g][n] = 0.0
    for g in range(1, MAX_STRIPS + 1):
        for i in range(n - 1, -1, -1):
            best, bj = INF, None
            for j in range(i + 1, n + 1):
                c = (j - i) * W[i] + dp[g - 1][j]
                if c < best:
                    best, bj = c, j
            if best < dp[g][i]:
                dp[g][i] = best
                nxt[g][i] = bj
            if dp[g - 1][i] < dp[g][i]:
                dp[g][i] = dp[g - 1][i]
                nxt[g][i] = nxt[g - 1][i]
    strips = []
    i, g = 0, MAX_STRIPS
    while i < n:
        while g > 0 and dp[g - 1][i] == dp[g][i]:
            g -= 1
        j = nxt[g][i]
        strips.append((j - i, W[i]))
        i = j
        g -= 1
    return strips


def _plan(strips):
    """Half split (blend overlap) + DMA groups (strip-aligned, ~GROUP_ELEMS
    per plane, group boundary forced at the half cut).  Returns (cut, groups)
    where groups = list of (strip_lo, strip_hi, plane_elems)."""
    pre, cut = 0, 0
    for gi, (sp, W) in enumerate(strips):
        if pre + sp > SP // 2 and cut == 0:
            cut = gi if abs(pre - SP // 2) <= abs(pre + sp - SP // 2) else gi + 1
        pre += sp
    cut = max(1, min(cut, len(strips) - 1))

    groups = []
    for lo, hi in ((0, cut), (cut, len(strips))):
        q0 = lo
        acc = 0
        for s in range(lo, hi):
            sz = strips[s][0] * strips[s][1]
            if acc and acc + sz > GROUP_ELEMS:
                groups.append((q0, s, acc))
                q0, acc = s, 0
            acc += sz
        groups.append((q0, hi, acc))
    return cut, groups


def _build_program(strips, repeat=None):
    import contextlib
    import concourse.bass as bass
    import concourse.mybir as mybir
    from concourse import tile

    f32 = mybir.dt.float32
    f16 = mybir.dt.float16
    alu = mybir.AluOpType
    act_copy = mybir.ActivationFunctionType.Copy

    cut, groups = _plan(strips)
    TOT = sum(sp * W for sp, W in strips)
    sp_pre = sum(sp for sp, _ in strips[:cut])
    halves = [(0, cut, 0, sp_pre), (cut, len(strips), sp_pre, SP - sp_pre)]

    nc = bass.Bass()
    rect = nc.declare_dram_parameter("rect", [P, 3 * TOT], f16, isOutput=False)
    wth = nc.declare_dram_parameter("wth", [P, 4], f32, isOutput=False)
    cst = nc.declare_dram_parameter("cst", [P, CST_W], f32, isOutput=False)
    y = nc.declare_dram_parameter("y", [P, SP], f32, isOutput=True)

    with tile.TileContext(nc) as tc:
        with tc.tile_pool(name="const", bufs=1) as const, \
             tc.tile_pool(name="grp", bufs=1) as grp_pool, \
             tc.tile_pool(name="small", bufs=1) as small, \
             (tc.For_i(0, repeat, 1) if repeat else contextlib.nullcontext()):
            # tiny early DMA: W_theta, so compute starts fast
            wth_t = const.tile([P, 4], f32)
            nc.sync.dma_start(wth_t[:], wth[:])
            wt0, wt1, wt2 = wth_t[:, 0:1], wth_t[:, 1:2], wth_t[:, 2:3]

            # big consts — only needed for the blends
            cst_t = const.tile([P, CST_W], f32)
            nc.sync.dma_start(cst_t[:], cst[:])
            wp_ap = cst_t[:, 0:128]
            o = 131
            nx_ap = cst_t[:, o:o + SP]
            ny_ap = cst_t[:, o + SP:o + 2 * SP]
            nz_ap = cst_t[:, o + 2 * SP:o + 3 * SP]
            pv_ap = cst_t[:, o + 3 * SP:o + 4 * SP]

            smax_h = [const.tile([P, hsp], f16, tag=f"smax{h}",
                                 name=f"smax{h}")
                      for h, (_, _, _, hsp) in enumerate(halves)]
            smin_h = [const.tile([P, hsp], f16, tag=f"smin{h}",
                                 name=f"smin{h}")
                      for h, (_, _, _, hsp) in enumerate(halves)]

            c2 = None
            sn = None

            def emit_consts():
                nonlocal c2, sn
                c2r = const.tile([P, 1], f32, name="c2r")
                nc.vector.reduce_sum(c2r[:], wp_ap, axis=mybir.AxisListType.X)
                c2 = const.tile([P, 1], f32, name="c2")
                nc.vector.tensor_scalar_mul(c2[:], c2r[:], 0.5 / 128.0)
                sn_a = const.tile([P, SP], f32, name="sn_a")
                nc.vector.tensor_scalar_mul(sn_a[:], nx_ap, wt0)
                sn_b = const.tile([P, SP], f32, name="sn_b")
                nc.vector.scalar_tensor_tensor(
                    sn_b[:], ny_ap, wt1, sn_a[:], op0=alu.mult, op1=alu.add)
                sn = const.tile([P, SP], f32, name="sn")
                nc.vector.scalar_tensor_tensor(
                    sn[:], nz_ap, wt2, sn_b[:], op0=alu.mult, op1=alu.add)

            def emit_blend(h):
                _, _, n0, hsp = halves[h]
                sl = slice(n0, n0 + hsp)
                a_t = small.tile([P, hsp], f32, tag=f"a{h}", name=f"a{h}")
                nc.vector.scalar_tensor_tensor(
                    a_t[:], smin_h[h][:], -1.0, sn[:, sl],
                    op0=alu.mult, op1=alu.add)
                b_t = small.tile([P, hsp], f32, tag=f"b{h}", name=f"b{h}")
                nc.vector.scalar_tensor_tensor(
                    b_t[:], sn[:, sl], -1.0, smax_h[h][:],
                    op0=alu.mult, op1=alu.add)
                md = small.tile([P, hsp], f32, tag=f"md{h}", name=f"md{h}")
                nc.vector.tensor_max(md[:], a_t[:], b_t[:])
                md2 = small.tile([P, hsp], f32, tag=f"md2{h}", name=f"md2{h}")
                nc.vector.tensor_scalar(
                    md2[:], md[:], 0.0, c2[:], op0=alu.max, op1=alu.mult)
                y_c = small.tile([P, hsp], f32, tag=f"y{h}", name=f"y{h}")
                nc.vector.scalar_tensor_tensor(
                    y_c[:], pv_ap[:, sl], 0.5, md2[:],
                    op0=alu.mult, op1=alu.add)
                nc.sync.dma_start(y[:, sl], y_c[:])

            # ---- stream groups: one DMA each, all math on DVE, in place ----
            rect_off = 0          # elem offset of group in rect
            node_off = 0          # node-slot offset of strip
            gi = 0
            for h, (g0, g1, n0, hsp) in enumerate(halves):
                while gi < len(groups) and groups[gi][0] < g1:
                    s_lo, s_hi, gsz = groups[gi]
                    grp_t = grp_pool.tile([P, 3 * gsz], f16,
                                          tag=f"grp{gi}", name=f"grp{gi}")
                    dma_eng = nc.sync if gi % 2 == 0 else nc.scalar
                    dma_eng.dma_start(
                        grp_t[:], rect[:, rect_off:rect_off + 3 * gsz])
                    rxr = grp_t[:, 0:gsz]
                    ryr = grp_t[:, gsz:2 * gsz]
                    rzr = grp_t[:, 2 * gsz:3 * gsz]
                    # u = rx*w0 + ry*w1 + rz*w2, computed in place
                    nc.vector.tensor_scalar_mul(rxr, rxr, wt0)
                    nc.vector.tensor_scalar_mul(ryr, ryr, wt1)
                    nc.vector.tensor_scalar_mul(rzr, rzr, wt2)
                    nc.vector.tensor_add(ryr, rxr, ryr)
                    nc.vector.tensor_add(rzr, ryr, rzr)
                    # per-strip segmented min/max into the half accumulators
                    sbase = 0
                    for s in range(s_lo, s_hi):
                        sp, W = strips[s]
                        u3 = grp_t[:, 2 * gsz + sbase:2 * gsz + sbase + sp * W]
                        u3 = u3.rearrange("p (s d) -> p s d", d=W)
                        sl_n = slice(node_off - n0, node_off - n0 + sp)
                        nc.vector.tensor_reduce(
                            smax_h[h][:, sl_n], u3,
                            axis=mybir.AxisListType.X, op=alu.max)
                        nc.vector.tensor_reduce(
                            smin_h[h][:, sl_n], u3,
                            axis=mybir.AxisListType.X, op=alu.min)
                        sbase += sp * W
                        node_off += sp
                    rect_off += 3 * gsz
                    gi += 1
                if h == 0:
                    emit_consts()
                emit_blend(h)
    return _legalize_waits(nc)


def _host_layout(previous_inclusion_score, nodes, row_indices, col_indices,
                 W_phi, W_theta):
    """Pure data-movement prep: route edges to cores by destination-node
    range, degree-sort nodes within each core, pack neighborhoods into
    degree strips, pack strip planes into per-group [RX|RY|RZ] blocks."""
    prev = np.ascontiguousarray(np.asarray(previous_inclusion_score, np.float32))
    nodes = np.ascontiguousarray(np.asarray(nodes, np.float32))
    rows = np.asarray(row_indices).astype(np.int64, copy=False)
    cols = np.asarray(col_indices).astype(np.int64, copy=False)
    wphi = np.asarray(W_phi, np.float32).reshape(-1)
    wtheta = np.asarray(W_theta, np.float32).reshape(-1)
    nodes16 = nodes.astype(np.float16)

    order = np.argsort(rows, kind="stable")
    rs = rows[order]
    cs = cols[order]
    bounds = np.searchsorted(rs, np.arange(N_NODES + 1))
    start_all = bounds[:-1]
    deg_all = bounds[1:] - bounds[:-1]

    # per-core degree-rank permutation; global strip widths
    core_order = []
    core_sdeg = []
    for k in range(N_CORES):
        dk = np.zeros(NPAD, np.int64)
        dk[:NPC] = deg_all[k * NPC:(k + 1) * NPC]
        ordk = np.argsort(-dk, kind="stable")
        core_order.append(ordk)
        core_sdeg.append(dk[ordk])
    sdeg = np.max(np.stack(core_sdeg), axis=0)
    degmax_sp = sdeg[::P][:SP]
    strips = tuple(_choose_strips(degmax_sp))
    cut, groups = _plan(strips)
    TOT = sum(sp * W for sp, W in strips)

    # rank <-> flat-slot map (strip-blocked layout; see _build_program)
    rank_of_slot = np.empty(NPAD, np.int64)
    off, r0 = 0, 0
    for sp, W in strips:
        pp = np.arange(P)[:, None]
        jj = np.arange(sp)[None, :]
        rank_of_slot[(pp * SP + off + jj).ravel()] = (r0 + pp * sp + jj).ravel()
        off += sp
        r0 += sp * P

    in_maps = [dict() for _ in range(N_CORES)]
    for k in range(N_CORES):
        lo = k * NPC
        ordk = core_order[k]
        nid = np.where(ordk < NPC, lo + ordk, lo)
        deg_r = np.where(ordk < NPC,
                         deg_all[np.minimum(lo + ordk, N_NODES - 1)], 0)
        start_r = start_all[nid]

        # per-strip planes (rank order)
        sxyz = []
        roff = 0
        for sp, W in strips:
            n_strip = sp * P
            nid_g = nid[roff:roff + n_strip]
            deg_g = deg_r[roff:roff + n_strip]
            start_g = start_r[roff:roff + n_strip]
            offs = np.minimum(np.arange(W)[None, :],
                              np.maximum(deg_g[:, None] - 1, 0))
            idx = start_g[:, None] + offs
            np.clip(idx, 0, N_EDGES - 1, out=idx)
            col_rect = cs[idx]
            empty = deg_g == 0
            if empty.any():
                col_rect[empty, :] = nid_g[empty, None]
            planes = nodes16[col_rect]  # [n_strip, W, 3]
            sxyz.append(tuple(
                np.ascontiguousarray(planes[:, :, c].reshape(P, sp * W))
                for c in range(3)))
            roff += n_strip

        rect = np.empty((P, 3 * TOT), np.float16)
        rect_off = 0
        for s_lo, s_hi, gsz in groups:
            for c in range(3):
                b = rect_off + c * gsz
                for s in range(s_lo, s_hi):
                    w = strips[s][0] * strips[s][1]
                    rect[:, b:b + w] = sxyz[s][c]
                    b += w
            rect_off += 3 * gsz
        in_maps[k]["rect"] = rect

        own = nodes[nid]
        own[ordk >= NPC] = 0.0
        pvk = prev[np.minimum(nid, N_NODES - 1)]
        pvk[ordk >= NPC] = 0.0
        own_s = own[rank_of_slot]
        pvk_s = pvk[rank_of_slot]
        cstk = np.empty((P, CST_W), np.float32)
        cstk[:, 0:128] = wphi[None, :]
        cstk[:, 128:131] = wtheta[None, :]
        o = 131
        cstk[:, o:o + SP] = own_s[:, 0].reshape(P, SP)
        cstk[:, o + SP:o + 2 * SP] = own_s[:, 1].reshape(P, SP)
        cstk[:, o + 2 * SP:o + 3 * SP] = own_s[:, 2].reshape(P, SP)
        cstk[:, o + 3 * SP:o + 4 * SP] = pvk_s.reshape(P, SP)
        in_maps[k]["cst"] = cstk
        wthk = np.zeros((P, 4), np.float32)
        wthk[:, 0:3] = wtheta[None, :]
        in_maps[k]["wth"] = wthk
    return in_maps, strips, core_order, rank_of_slot


def kernel(previous_inclusion_score, nodes, row_indices, col_indices,
           W_phi, W_theta, _trace=False):
    global LAST_RESULTS
    in_maps, strips, core_order, rank_of_slot = _host_layout(
        previous_inclusion_score, nodes, row_indices, col_indices,
        W_phi, W_theta)
    if strips not in _prog_cache:
        _prog_cache[strips] = _build_program(strips)
    nc = _prog_cache[strips]

    from concourse.bass_utils import run_bass_kernel_spmd
    res = run_bass_kernel_spmd(nc, in_maps, list(range(N_CORES)), trace=_trace)
    LAST_RESULTS = res

    out = np.empty(N_NODES, np.float32)
    for k in range(N_CORES):
        y_flat = np.asarray(res.results[k]["y"]).reshape(NPAD)
        y_rank = np.empty(NPAD, np.float32)
        y_rank[rank_of_slot] = y_flat
        y_slot = np.empty(NPAD, np.float32)
        y_slot[core_order[k]] = y_rank
        out[k * NPC:(k + 1) * NPC] = y_slot[:NPC]
    return out

